# revision 11
# baseline (speedup 1.0000x reference)
"""GF(2) linear block encoder c = (b @ G) mod 2 on 8 TRN2 NeuronCores.

Strategy:
  - Data-parallel: shard b rows (32768 -> 8 x 4096), replicate G.
  - Bits {0,1} are exact in fp8-e4m3 and products accumulate exactly in
    fp32 PSUM, so the GF(2) matmul is an fp8 DoubleRow matmul (K=256 per
    MM) at 2x bf16 throughput -- the PE floor for this shape (~110us).
  - Output is written as uint8 bits (mod-2 extracted from PSUM by the
    DVE/Pool engines) and upcast to int32 on the host: 4x less output
    HBM traffic than int32, which removes the output-DMA tail.
  - Input DMAs are ordered so the first matmul's operands (G k-pair 0,
    b chunk 0) land first; the matmul loop is k-outer per m-tile so PE
    starts as soon as those arrive instead of after all 6 MiB of input.
"""

import sys

import numpy as np

if "/opt/trn_rl_repo" not in sys.path:
    sys.path.insert(0, "/opt/trn_rl_repo")

import ml_dtypes

B_ROWS = 32768
K_MSG = 1024
N_CODE = 2048
NCORES = 8
M = B_ROWS // NCORES  # 4096 rows per core
KS = K_MSG // 128     # 8 k-subtiles of 128
KP = KS // 2          # 4 DoubleRow k-pair steps (K=256 each)
MT = M // 128         # 32 m-tiles
NT = N_CODE // 512    # 4 n-chunks (one PSUM bank each)
MC = 16               # b DMA chunks along m (2 m-tiles each)
MCW = M // MC         # 256 rows per chunk

F8 = ml_dtypes.float8_e4m3

_NC_CACHE = None


def _build_bass():
    import concourse.bacc as bacc
    import concourse.mybir as mybir
    from concourse import tile

    nc = bacc.Bacc("TRN2", target_bir_lowering=False, debug=False)

    # bt[p, c, s, j] = b bit for row m = c*MCW + j, k = s*128 + p
    bt = nc.dram_tensor("bt", [128, MC, KS, MCW], mybir.dt.float8e4, kind="ExternalInput")
    g = nc.dram_tensor("g", [128, KS, N_CODE], mybir.dt.float8e4, kind="ExternalInput")
    c = nc.dram_tensor("c", [M, N_CODE], mybir.dt.uint16, kind="ExternalOutput")

    dr = mybir.MatmulPerfMode.DoubleRow

    with tile.TileContext(nc) as tc:
        with (
            tc.tile_pool(name="persist", bufs=1) as persist,
            tc.tile_pool(name="psum", bufs=2, space="PSUM") as psum_pool,
            tc.tile_pool(name="mids", bufs=4) as mids,
        ):
            # --- input DMAs, ordered for earliest PE start ---
            # sync:   g0, b0, g2, then odd b chunks
            # scalar: g1, g3, then even b chunks
            g_tiles = [
                persist.tile([128, 2, N_CODE], mybir.dt.float8e4, name=f"gt{kp}", tag=f"g{kp}")
                for kp in range(KP)
            ]
            b_tiles = [
                persist.tile([128, KS, MCW], mybir.dt.float8e4, name=f"btile{mc}", tag=f"b{mc}")
                for mc in range(MC)
            ]

            NH = N_CODE // 2

            def load_b(mc, eng):
                eng.dma_start(out=b_tiles[mc], in_=bt[:, mc, :, :])

            # SWDGE: first b piece (k-pair 0 of chunk 0 -> 64 KiB) so the
            # first matmul can start on it + g0; then early b chunks.
            nc.gpsimd.dma_start(out=b_tiles[0][:, 0:2, :], in_=bt[:, 0, 0:2, :])
            nc.gpsimd.dma_start(out=b_tiles[0][:, 2:KS, :], in_=bt[:, 0, 2:KS, :])
            for mc in (1, 2, 3):
                load_b(mc, nc.gpsimd)
            # HWDGE queues: G halves staggered (kp order) so g_kp completes
            # every ~2us; then the remaining b chunks.
            for kp in range(KP):
                nc.sync.dma_start(
                    out=g_tiles[kp][:, :, 0:NH], in_=g[:, 2 * kp : 2 * kp + 2, 0:NH]
                )
                nc.scalar.dma_start(
                    out=g_tiles[kp][:, :, NH:], in_=g[:, 2 * kp : 2 * kp + 2, NH:]
                )
            rr = [nc.sync, nc.scalar]
            for mc in range(4, MC):
                load_b(mc, rr[mc % 2])

            # PE p-state pre-warm: dummy matmuls on zeroed tiles while the
            # first inputs stream in, so real matmuls start at full clock
            zb = persist.tile([128, 2, 128], mybir.dt.float8e4, name="zwarm")
            nc.vector.memset(zb, 0)
            ps_warm = psum_pool.tile([128, N_CODE], mybir.dt.float32, name="ps")
            for w in range(12):
                nc.tensor.matmul(
                    ps_warm[:, 0:128],
                    zb,
                    zb,
                    start=True,
                    stop=True,
                    perf_mode=dr,
                )

            # output viewed per m-tile: m = mt*128 + p
            c_view = c.rearrange("(mt p) n -> mt p n", p=128)

            # mod-2 = LSB: ACT casts PSUM fp32 -> uint16 SBUF (exact, sums
            # <= 1024), then DVE does an in-place and-with-1 (Pool lacks
            # tensor_scalar, and only ACT/DVE can read PSUM)
            ext_engines = [nc.vector, nc.vector]
            # out-DMA queues: early tiles on SWDGE (HWDGE queues still
            # carry inputs then), later tiles rotate over all three
            out_eng = [nc.gpsimd] * 6 + [
                (nc.gpsimd, nc.sync, nc.scalar)[i % 3] for i in range(MT - 6)
            ]

            def mm(ps, mt, kp, nt):
                mc, j = mt // 2, mt % 2
                nc.tensor.matmul(
                    ps[:, nt * 512 : (nt + 1) * 512],
                    b_tiles[mc][:, 2 * kp : 2 * kp + 2, j * 128 : (j + 1) * 128],
                    g_tiles[kp][:, :, nt * 512 : (nt + 1) * 512],
                    start=(kp == 0),
                    stop=(kp == KP - 1),
                    perf_mode=dr,
                )

            def extract(ps, mid, n0, n1, eng):
                nc.scalar.activation(
                    mid[:, n0:n1], ps[:, n0:n1], mybir.ActivationFunctionType.Copy
                )
                eng.tensor_scalar(
                    out=mid[:, n0:n1],
                    in0=mid[:, n0:n1],
                    scalar1=1,
                    scalar2=None,
                    op0=mybir.AluOpType.bitwise_and,
                )

            # tiles 0,1: kp-outer across the PAIR (shared b chunk 0) to
            # double the slack for staggered G arrival
            ps_pair = [
                psum_pool.tile([128, N_CODE], mybir.dt.float32, name="ps")
                for i in range(2)
            ]
            for kp in range(KP):
                for i in range(2):
                    for nt in range(NT):
                        mm(ps_pair[i], i, kp, nt)
            for i in range(2):
                mid = mids.tile([128, N_CODE], mybir.dt.uint16)
                extract(ps_pair[i], mid, 0, N_CODE, ext_engines[i % 2])
                out_eng[i].dma_start(out=c_view[i], in_=mid)

            for mt in range(2, MT):
                ps = psum_pool.tile([128, N_CODE], mybir.dt.float32, name="ps")  # 4 banks
                if mt < MT - 1:
                    for kp in range(KP):
                        for nt in range(NT):
                            mm(ps, mt, kp, nt)
                    mid = mids.tile([128, N_CODE], mybir.dt.uint16)
                    extract(ps, mid, 0, N_CODE, ext_engines[mt % 2])
                    out_eng[mt].dma_start(out=c_view[mt], in_=mid)
                else:
                    # last tile: nt-outer so each 512-col chunk extracts and
                    # streams out while the PE finishes the later chunks
                    mid = mids.tile([128, N_CODE], mybir.dt.uint16)
                    for nt in range(NT):
                        for kp in range(KP):
                            mm(ps, mt, kp, nt)
                        extract(ps, mid, nt * 512, (nt + 1) * 512, ext_engines[nt % 2])
                        out_eng[mt].dma_start(
                            out=c_view[mt][:, nt * 512 : (nt + 1) * 512],
                            in_=mid[:, nt * 512 : (nt + 1) * 512],
                        )

    nc.finalize()
    return nc


def _get_nc():
    global _NC_CACHE
    if _NC_CACHE is None:
        _NC_CACHE = _build_bass()
    return _NC_CACHE


def _pack_inputs(b, G):
    b8 = np.asarray(b).astype(np.uint8)
    G8 = np.asarray(G).astype(np.uint8)
    # g[p, s, n], k = s*128 + p
    g_f8 = G8.reshape(KS, 128, N_CODE).transpose(1, 0, 2).astype(F8, order="C")
    bts = []
    for core in range(NCORES):
        sh = b8[core * M : (core + 1) * M]  # [M, K]
        # bt[p, c, s, j]: m = c*MCW + j, k = s*128 + p
        btc = sh.reshape(MC, MCW, KS, 128).transpose(3, 0, 2, 1)
        bts.append(btc.astype(F8, order="C"))
    return bts, g_f8


def kernel(b, G, trace=False, **run_kwargs):
    from concourse.bass_utils import run_bass_kernel_spmd

    nc = _get_nc()
    bts, g_f8 = _pack_inputs(b, G)
    in_maps = [{"bt": bts[i], "g": g_f8} for i in range(NCORES)]
    res = run_bass_kernel_spmd(
        nc, in_maps, core_ids=list(range(NCORES)), trace=trace, **run_kwargs
    )
    out = np.concatenate([res.results[i]["c"] for i in range(NCORES)], axis=0)
    out = out.astype(np.int32)
    if trace:
        kernel.last_results = res
    return out


kernel.last_results = None


# revision 12
# speedup vs baseline: 1.0475x; 1.0475x over previous
"""GF(2) linear block encoder c = (b @ G) mod 2 on 8 TRN2 NeuronCores.

Strategy:
  - Data-parallel: shard b rows (32768 -> 8 x 4096), replicate G.
  - Bits {0,1} are exact in fp8-e4m3 and products accumulate exactly in
    fp32 PSUM, so the GF(2) matmul is an fp8 DoubleRow matmul (K=256 per
    MM) at 2x bf16 throughput -- the PE floor for this shape (~110us).
  - Output is written as uint8 bits (mod-2 extracted from PSUM by the
    DVE/Pool engines) and upcast to int32 on the host: 4x less output
    HBM traffic than int32, which removes the output-DMA tail.
  - Input DMAs are ordered so the first matmul's operands (G k-pair 0,
    b chunk 0) land first; the matmul loop is k-outer per m-tile so PE
    starts as soon as those arrive instead of after all 6 MiB of input.
"""

import sys

import numpy as np

if "/opt/trn_rl_repo" not in sys.path:
    sys.path.insert(0, "/opt/trn_rl_repo")

import ml_dtypes

B_ROWS = 32768
K_MSG = 1024
N_CODE = 2048
NCORES = 8
M = B_ROWS // NCORES  # 4096 rows per core
KS = K_MSG // 128     # 8 k-subtiles of 128
KP = KS // 2          # 4 DoubleRow k-pair steps (K=256 each)
MT = M // 128         # 32 m-tiles
NT = N_CODE // 512    # 4 n-chunks (one PSUM bank each)
MC = 16               # b DMA chunks along m (2 m-tiles each)
MCW = M // MC         # 256 rows per chunk

F8 = ml_dtypes.float8_e4m3

_NC_CACHE = None


def _build_bass():
    import concourse.bacc as bacc
    import concourse.mybir as mybir
    from concourse import tile

    nc = bacc.Bacc("TRN2", target_bir_lowering=False, debug=False)

    # bt[p, c, s, j] = b bit for row m = c*MCW + j, k = s*128 + p
    bt = nc.dram_tensor("bt", [128, MC, KS, MCW], mybir.dt.float8e4, kind="ExternalInput")
    g = nc.dram_tensor("g", [128, KS, N_CODE], mybir.dt.float8e4, kind="ExternalInput")
    c = nc.dram_tensor("c", [M, N_CODE], mybir.dt.uint16, kind="ExternalOutput")

    dr = mybir.MatmulPerfMode.DoubleRow

    with tile.TileContext(nc) as tc:
        with (
            tc.tile_pool(name="persist", bufs=1) as persist,
            tc.tile_pool(name="psum", bufs=2, space="PSUM") as psum_pool,
            tc.tile_pool(name="mids", bufs=4) as mids,
        ):
            # --- input DMAs, ordered for earliest PE start ---
            # sync:   g0, b0, g2, then odd b chunks
            # scalar: g1, g3, then even b chunks
            g_tiles = [
                persist.tile([128, 2, N_CODE], mybir.dt.float8e4, name=f"gt{kp}", tag=f"g{kp}")
                for kp in range(KP)
            ]
            b_tiles = [
                persist.tile([128, KS, MCW], mybir.dt.float8e4, name=f"btile{mc}", tag=f"b{mc}")
                for mc in range(MC)
            ]

            NH = N_CODE // 2

            def load_b(mc, eng):
                eng.dma_start(out=b_tiles[mc], in_=bt[:, mc, :, :])

            # Inputs on the two HWDGE queues only (SWDGE delivers late).
            # DMA lines spray across all 16 engines, so global queue order
            # is what matters: critical first-matmul pieces strictly first.
            def load_g_half(kp, half, eng):
                eng.dma_start(
                    out=g_tiles[kp][:, :, half * NH : (half + 1) * NH],
                    in_=g[:, 2 * kp : 2 * kp + 2, half * NH : (half + 1) * NH],
                )

            nc.sync.dma_start(out=b_tiles[0][:, 0:2, :], in_=bt[:, 0, 0:2, :])
            load_g_half(0, 0, nc.sync)
            load_g_half(0, 1, nc.scalar)
            load_g_half(1, 0, nc.sync)
            load_g_half(1, 1, nc.scalar)
            nc.scalar.dma_start(out=b_tiles[0][:, 2:KS, :], in_=bt[:, 0, 2:KS, :])
            load_b(1, nc.sync)
            load_g_half(2, 0, nc.sync)
            load_g_half(2, 1, nc.scalar)
            load_g_half(3, 0, nc.sync)
            load_g_half(3, 1, nc.scalar)
            load_b(2, nc.scalar)
            load_b(3, nc.sync)
            rr = [nc.scalar, nc.sync]
            for mc in range(4, MC):
                load_b(mc, rr[mc % 2])

            # PE p-state pre-warm: dummy matmuls on zeroed tiles while the
            # first inputs stream in, so real matmuls start at full clock
            zb = persist.tile([128, 2, 128], mybir.dt.float8e4, name="zwarm")
            nc.vector.memset(zb, 0)
            ps_warm = psum_pool.tile([128, N_CODE], mybir.dt.float32, name="ps")
            for w in range(18):
                nc.tensor.matmul(
                    ps_warm[:, 0:128],
                    zb,
                    zb,
                    start=True,
                    stop=True,
                    perf_mode=dr,
                )

            # output viewed per m-tile: m = mt*128 + p
            c_view = c.rearrange("(mt p) n -> mt p n", p=128)

            # mod-2 = LSB: ACT casts PSUM fp32 -> uint16 SBUF (exact, sums
            # <= 1024), then DVE does an in-place and-with-1 (Pool lacks
            # tensor_scalar, and only ACT/DVE can read PSUM)
            ext_engines = [nc.vector, nc.vector]
            # out-DMA queues rotate over all three (inputs are queued
            # ahead on the HWDGE queues and drain first)
            out_eng = [(nc.gpsimd, nc.sync, nc.scalar)[i % 3] for i in range(MT)]

            def mm(ps, mt, kp, nt):
                mc, j = mt // 2, mt % 2
                nc.tensor.matmul(
                    ps[:, nt * 512 : (nt + 1) * 512],
                    b_tiles[mc][:, 2 * kp : 2 * kp + 2, j * 128 : (j + 1) * 128],
                    g_tiles[kp][:, :, nt * 512 : (nt + 1) * 512],
                    start=(kp == 0),
                    stop=(kp == KP - 1),
                    perf_mode=dr,
                )

            def extract(ps, mid, n0, n1, eng):
                nc.scalar.activation(
                    mid[:, n0:n1], ps[:, n0:n1], mybir.ActivationFunctionType.Copy
                )
                eng.tensor_scalar(
                    out=mid[:, n0:n1],
                    in0=mid[:, n0:n1],
                    scalar1=1,
                    scalar2=None,
                    op0=mybir.AluOpType.bitwise_and,
                )

            # tiles 0,1: kp-outer across the PAIR (shared b chunk 0) to
            # double the slack for staggered G arrival
            ps_pair = [
                psum_pool.tile([128, N_CODE], mybir.dt.float32, name="ps")
                for i in range(2)
            ]
            for kp in range(KP):
                for i in range(2):
                    for nt in range(NT):
                        mm(ps_pair[i], i, kp, nt)
            for i in range(2):
                mid = mids.tile([128, N_CODE], mybir.dt.uint16)
                extract(ps_pair[i], mid, 0, N_CODE, ext_engines[i % 2])
                out_eng[i].dma_start(out=c_view[i], in_=mid)

            for mt in range(2, MT):
                ps = psum_pool.tile([128, N_CODE], mybir.dt.float32, name="ps")  # 4 banks
                if mt < MT - 1:
                    for kp in range(KP):
                        for nt in range(NT):
                            mm(ps, mt, kp, nt)
                    mid = mids.tile([128, N_CODE], mybir.dt.uint16)
                    extract(ps, mid, 0, N_CODE, ext_engines[mt % 2])
                    out_eng[mt].dma_start(out=c_view[mt], in_=mid)
                else:
                    # last tile in two n-halves so extraction and out-DMA of
                    # the first half overlap the PE's second half
                    mid = mids.tile([128, N_CODE], mybir.dt.uint16)
                    for h in range(2):
                        for kp in range(KP):
                            for nt in (2 * h, 2 * h + 1):
                                mm(ps, mt, kp, nt)
                        extract(ps, mid, h * NH, (h + 1) * NH, ext_engines[h % 2])
                        out_eng[mt].dma_start(
                            out=c_view[mt][:, h * NH : (h + 1) * NH],
                            in_=mid[:, h * NH : (h + 1) * NH],
                        )

    nc.finalize()
    return nc


def _get_nc():
    global _NC_CACHE
    if _NC_CACHE is None:
        _NC_CACHE = _build_bass()
    return _NC_CACHE


def _pack_inputs(b, G):
    b8 = np.asarray(b).astype(np.uint8)
    G8 = np.asarray(G).astype(np.uint8)
    # g[p, s, n], k = s*128 + p
    g_f8 = G8.reshape(KS, 128, N_CODE).transpose(1, 0, 2).astype(F8, order="C")
    bts = []
    for core in range(NCORES):
        sh = b8[core * M : (core + 1) * M]  # [M, K]
        # bt[p, c, s, j]: m = c*MCW + j, k = s*128 + p
        btc = sh.reshape(MC, MCW, KS, 128).transpose(3, 0, 2, 1)
        bts.append(btc.astype(F8, order="C"))
    return bts, g_f8


def kernel(b, G, trace=False, **run_kwargs):
    from concourse.bass_utils import run_bass_kernel_spmd

    nc = _get_nc()
    bts, g_f8 = _pack_inputs(b, G)
    in_maps = [{"bt": bts[i], "g": g_f8} for i in range(NCORES)]
    res = run_bass_kernel_spmd(
        nc, in_maps, core_ids=list(range(NCORES)), trace=trace, **run_kwargs
    )
    out = np.concatenate([res.results[i]["c"] for i in range(NCORES)], axis=0)
    out = out.astype(np.int32)
    if trace:
        kernel.last_results = res
    return out


kernel.last_results = None
